# revision 4
# baseline (speedup 1.0000x reference)
"""Trainium2 Bass kernel for nn_FDLT (forward discrete Legendre transform).

Math: for each of the 127 m-blocks, the reference does
    out[:, mi, :] = (Cm[mi] * psiHat[:, mi, :]) @ XF_mi @ Dblk_mi.T
where XF_mi alternates XFc/XFs by mi parity and Dblk_mi is the mi-th
block of the block-diagonal sparse Wigner matrix D.  All tables are
runtime constants, so fold them on the host into A_mi = Cm[mi] * XF_mi
@ Dblk_mi.T (shape [128, 64]) and the device work collapses to 127
independent [512,128]@[128,64] matmuls.

Sharding: m-parallel across 8 cores (16 blocks/core, padded 128 with a
zero block), full batch per core.  The host feeds each core its input
slab pre-transposed to [n, j, b] so the contraction dim n lands on the
SBUF partition axis; the tensor engine computes out_t[l, b] per block
(lhsT = A_mi stationary, moving rhs = [128, 512]).  Block pairs share
one PSUM bank via column tiling so copies/stores run at the full 128
partitions.
"""

import numpy as np

import concourse.bacc as bacc
import concourse.bass as bass  # noqa: F401
import concourse.mybir as mybir
from concourse import tile
from concourse.bass_utils import run_bass_kernel_spmd

P = 128      # SBUF partitions = n dim (2B)
B = 64       # l dim per block
M = 127      # number of m blocks
NB = 512     # full batch
NCORES = 8
JPC = 16     # m-blocks per core (8*16 = 128 = 127 real + 1 zero pad)
PAIRS = JPC // 2
SLAB = 4     # m-blocks per input DMA (1.05 MB slabs)

DT_IN = mybir.dt.float32  # device input dtype for xt/av

_programs = {}


def _build(dt_in):
    nc = bacc.Bacc(
        "TRN2", target_bir_lowering=False, debug=False, num_devices=NCORES
    )
    xt = nc.dram_tensor("xt", [P, JPC * NB], dt_in, kind="ExternalInput")
    av = nc.dram_tensor("av", [P, JPC * B], dt_in, kind="ExternalInput")
    out = nc.dram_tensor(
        "out", [P, PAIRS * NB], mybir.dt.float32, kind="ExternalOutput"
    )
    with tile.TileContext(nc) as tc:
        with (
            tc.tile_pool(name="cpool", bufs=1) as cpool,
            tc.tile_pool(name="xpool", bufs=3) as xpool,
            tc.tile_pool(name="ppool", bufs=4, space="PSUM") as ppool,
            tc.tile_pool(name="opool", bufs=3) as opool,
        ):
            a_sb = cpool.tile([P, JPC * B], dt_in)
            nc.sync.dma_start(out=a_sb[:], in_=av[:])
            for s in range(JPC // SLAB):
                x_sb = xpool.tile([P, SLAB * NB], dt_in)
                nc.sync.dma_start(
                    out=x_sb[:], in_=xt[:, s * SLAB * NB : (s + 1) * SLAB * NB]
                )
                for q in range(SLAB // 2):
                    j0 = s * SLAB + 2 * q
                    ps = ppool.tile([P, NB], mybir.dt.float32)
                    nc.tensor.matmul(
                        ps[0:B, :],
                        lhsT=a_sb[:, j0 * B : (j0 + 1) * B],
                        rhs=x_sb[:, (2 * q) * NB : (2 * q + 1) * NB],
                        start=True,
                        stop=True,
                    )
                    nc.tensor.matmul(
                        ps[B:P, :],
                        lhsT=a_sb[:, (j0 + 1) * B : (j0 + 2) * B],
                        rhs=x_sb[:, (2 * q + 1) * NB : (2 * q + 2) * NB],
                        start=True,
                        stop=True,
                        tile_position=(0, B),
                    )
                    o_sb = opool.tile([P, NB], mybir.dt.float32)
                    nc.vector.tensor_copy(o_sb[:], ps[:])
                    c = s * (SLAB // 2) + q
                    nc.sync.dma_start(out=out[:, c * NB : (c + 1) * NB], in_=o_sb[:])
    nc.compile()
    return nc


def _get_program(dt_in):
    key = str(dt_in)
    if key not in _programs:
        _programs[key] = _build(dt_in)
    return _programs[key]


def _fold_tables(Cm, XFc, XFs, D_val, D_row, D_col):
    """A[mi] = Cm[mi] * XF_mi @ Dblk_mi.T in float64 -> [128, 128, 64]."""
    Cm = np.asarray(Cm, np.float64)
    XFc = np.asarray(XFc, np.float64)
    XFs = np.asarray(XFs, np.float64)
    vals = np.asarray(D_val, np.float64)
    rows = np.asarray(D_row, np.int64)
    cols = np.asarray(D_col, np.int64)

    mi = rows // B
    l = rows - mi * B
    n = cols - mi * (2 * B)
    Dt = np.zeros((M, 2 * B, B))  # [mi, n, l] = Dblk_mi.T
    Dt[mi, n, l] = vals

    A = np.zeros((P, P, B))  # padded to 128 blocks; A[127] stays 0
    # B-1 = 63 is odd -> cos rows are the odd mi, sin rows the even mi
    A[0:M:2] = np.einsum("nk,mkl->mnl", XFs, Dt[0::2], optimize=True)
    A[1:M:2] = np.einsum("nk,mkl->mnl", XFc, Dt[1::2], optimize=True)
    A[:M] *= Cm[:, None, None]
    return A


def _np_dtype(dt_in):
    return mybir.dt.np(dt_in)


def _run(psiHat, A, trace=False, dt_in=DT_IN):
    dt_np = _np_dtype(dt_in)
    # [b, m, n] -> [m, n, b], contiguous
    PT = np.ascontiguousarray(psiHat.transpose(1, 2, 0).astype(np.float32))

    in_maps = []
    for k in range(NCORES):
        mi0 = JPC * k
        nj = min(JPC, M - mi0)
        xt_k = np.zeros((P, JPC, NB), dt_np)
        xt_k[:, :nj, :] = PT[mi0 : mi0 + nj].transpose(1, 0, 2)
        a_k = np.zeros((P, JPC, B), dt_np)
        a_k[:, :nj, :] = A[mi0 : mi0 + nj].transpose(1, 0, 2)
        in_maps.append(
            {"xt": xt_k.reshape(P, JPC * NB), "av": a_k.reshape(P, JPC * B)}
        )

    nc = _get_program(dt_in)
    res = run_bass_kernel_spmd(nc, in_maps, list(range(NCORES)), trace=trace)

    out = np.empty((NB, M, B), np.float32)
    for k in range(NCORES):
        mi0 = JPC * k
        nj = min(JPC, M - mi0)
        o = np.asarray(res.results[k]["out"]).reshape(2, B, PAIRS, NB)  # [h,l,p,b]
        ot = o.transpose(2, 0, 1, 3).reshape(JPC, B, NB)  # [j, l, b]
        out[:, mi0 : mi0 + nj, :] = ot[:nj].transpose(2, 0, 1)
    return out, res.exec_time_ns


def kernel(psiHat, Cm, XFc, XFs, D_val, D_row, D_col):
    psiHat = np.asarray(psiHat)
    A = _fold_tables(Cm, XFc, XFs, D_val, D_row, D_col)
    return _run(psiHat, A, trace=False)[0]
